# revision 5
# baseline (speedup 1.0000x reference)
"""ConditionalSigKerMMD discriminator loss on 8 TRN2 NeuronCores.

Strategy (self-contained, hardcoded for x,y,z of shape (64,33,8) fp32):
  - 4 signature-kernel Grams (K=rbf(x,x), Lgen=lin(y,y), Ltrue=lin(z,z),
    Lmix=lin(y,z)), each 64x64 pairs. Pair dim sharded 8 ways by row
    block (8 rows/core -> 512 pairs/gram/core, 2048 pairs/core total).
  - K gram via batched PE matmuls in (n,s)x(m,t) layout, bounced through
    DRAM into pair-major layout over both HW DGE queues (SP + Act).
  - L gram increments computed directly as Grams of differenced paths
    (inc = sum_d dY dZ for linear kernels) and bounced in fp16.
  - Goursat PDE solved as 64 row recurrences with tensor_tensor_scan:
    K (4 blocks) then L split 5 blocks on DVE + 7 with elementwise row
    math on Pool (scans stay on DVE; codegen rejects Pool scans).
  - K shards AllGathered; (K+I)^-1 via Newton-Schulz on PE/DVE/Act,
    emitted mid-solve so it overlaps the L recurrences; E = B - B^2;
    partial trace tr(E*Lsum) per core; host sums 8 scalars.

Execution path: the NEFF-wrapped executable and the constant inputs are
built once and cached; each call uploads one packed (3,64,33,8) buffer,
replicates/slices it on-device in a small prep jit, runs the Bass NEFF
on all 8 cores, and blocks on a single fetch of the 8 partial sums.
This keeps each call to ~one tunnel round-trip (the baseline rebuilt the
jit per call, paying re-trace + re-lower + extra round trips).
"""
import numpy as np

N_CORES = 8
N = 64           # pairs per gram row/col
S = 33           # path length
D = 8            # path dim
NL = 8           # rows (n values) per core
PQ = 64          # fine grid size (S-1)*2
CC = 32          # coarse coefficient grid
GRID = S * S     # 1089
SQRT192INV = 1.0 / np.sqrt(192.0)
NS_ITERS = 14

_CACHE = {}


def _build():
    import concourse.bass as bass
    import concourse.mybir as mybir
    import concourse.tile as tile
    import concourse.bacc as bacc

    f32 = mybir.dt.float32
    AX = mybir.AxisListType
    OP = mybir.AluOpType
    AF = mybir.ActivationFunctionType

    nc = bacc.Bacc("TRN2", target_bir_lowering=False, debug=False,
                   num_devices=N_CORES)

    # ---- I/O ----
    xf = nc.dram_tensor("xf", [N, S, D], f32, kind="ExternalInput").ap()
    yf = nc.dram_tensor("yf", [N, S, D], f32, kind="ExternalInput").ap()
    zf = nc.dram_tensor("zf", [N, S, D], f32, kind="ExternalInput").ap()
    xc = nc.dram_tensor("xc", [NL, S, D], f32, kind="ExternalInput").ap()
    yc = nc.dram_tensor("yc", [NL, S, D], f32, kind="ExternalInput").ap()
    zc = nc.dram_tensor("zc", [NL, S, D], f32, kind="ExternalInput").ap()
    eye_d = nc.dram_tensor("eye", [N, N], f32, kind="ExternalInput").ap()
    sel_d = nc.dram_tensor("sel", [N, NL], f32, kind="ExternalInput").ap()
    shf_d = nc.dram_tensor("shf", [128, N], f32, kind="ExternalInput").ap()
    out_d = nc.dram_tensor("out", [1, 1], f32, kind="ExternalOutput").ap()

    f16 = mybir.dt.float16

    # ---- internal DRAM ----
    # K gram bounces at full fp32 33x33 (exp comes before differencing);
    # L grams bounce only the 32x32 double-increments in fp16 (for linear
    # kernels inc[a,c] = sum_d dY[a,d]*dZ[c,d], computed directly from
    # differenced paths, so no post-exp differencing is needed).
    gdr0 = nc.dram_tensor("gram0", [NL, N, S, S], f32).ap()
    ldr = [nc.dram_tensor(f"linc{g}", [NL, N, CC, CC], f16).ap()
           for g in range(3)]
    x2c_dr = nc.dram_tensor("x2c", [NL, S], f32).ap()
    x2f_dr = nc.dram_tensor("x2f", [N, S], f32).ap()
    ksh_dr = nc.dram_tensor("ksh", [4, 128], f32).ap()
    kall_dr = nc.dram_tensor("kall", [N_CORES, 4, 2, N], f32,
                             addr_space="Shared").ap()

    # matmul chunking: n-row groups (output partitions <=128) x m chunks
    # (PSUM bank <=512 fp32)
    NCH = [(0, 3), (3, 3), (6, 2)]
    MCH = [(0, 15), (15, 15), (30, 15), (45, 15), (60, 4)]

    with tile.TileContext(nc) as tc:
        with (
            tc.tile_pool(name="cst", bufs=1) as cst,
            tc.tile_pool(name="lda", bufs=1) as lda,
            tc.tile_pool(name="mmp", bufs=4, space="PSUM") as mmp,
            tc.tile_pool(name="nsp", bufs=2, space="PSUM") as nsp,
            tc.tile_pool(name="tp", bufs=2, space="PSUM") as tpp,
            tc.tile_pool(name="gt", bufs=2) as gtp,
            tc.tile_pool(name="sc", bufs=2) as scp,
            tc.tile_pool(name="cf", bufs=2) as cfp,
            tc.tile_pool(name="pde", bufs=1) as pde,
            tc.tile_pool(name="rw", bufs=2) as rwp,
        ):
            # ============ loads & constants ============
            eye = lda.tile([N, N], f32, tag="eye")
            nc.sync.dma_start(eye[:], eye_d[:])
            sel = lda.tile([N, NL], f32, tag="sel")
            nc.sync.dma_start(sel[:], sel_d[:])
            shf = lda.tile([128, N], f32, tag="shf")
            nc.sync.dma_start(shf[:], shf_d[:])
            twoI = cst.tile([N, N], f32, tag="twoI")
            nc.vector.tensor_scalar_mul(twoI[:], eye[:], 2.0)
            ones64 = cst.tile([N, 1], f32, tag="ones64")
            nc.vector.memset(ones64[:], 1.0)
            onesrow = cst.tile([1, N], f32, tag="onesrow")
            nc.vector.memset(onesrow[:], 1.0)

            # B-side (rhs) tensors [D, (m,t)] and A-side (lhsT) [D, (n,s)].
            # x loads on the SP DMA queue, y/z on the Activation queue so the
            # startup ramp uses both HW DGE queues.
            bt = {}
            for nm, src in (("x", xf), ("y", yf), ("z", zf)):
                t = lda.tile([D, N * S], f32, tag=f"bt_{nm}")
                eng = nc.sync if nm == "x" else nc.scalar
                eng.dma_start(t[:], src.rearrange("m t d -> d (m t)"))
                bt[nm] = t
            at = {}
            for nm, src in (("x", xc), ("y", yc), ("z", zc)):
                t = lda.tile([D, NL * S], f32, tag=f"at_{nm}")
                eng = nc.sync if nm == "x" else nc.scalar
                eng.dma_start(t[:], src.rearrange("n s d -> d (n s)"))
                at[nm] = t

            # ============ x2 (squared norms) for rbf ============
            xsqc = lda.tile([S, NL * D], f32, tag="xsqc")
            nc.sync.dma_start(xsqc[:].rearrange("s (n d) -> s n d", n=NL),
                              xc.rearrange("n s d -> s n d"))
            sqc = lda.tile([S, NL * D], f32, tag="sqc")
            nc.scalar.activation(sqc[:], xsqc[:], AF.Square)
            x2c = lda.tile([S, NL], f32, tag="x2c")
            nc.vector.tensor_reduce(
                x2c[:], sqc[:].rearrange("s (n d) -> s n d", n=NL),
                axis=AX.X, op=OP.add)
            nc.sync.dma_start(x2c_dr.rearrange("n s -> s n"), x2c[:])

            xsqf = lda.tile([S, N * D], f32, tag="xsqf")
            nc.sync.dma_start(xsqf[:].rearrange("t (m d) -> t m d", m=N),
                              xf.rearrange("m t d -> t m d"))
            sqf = lda.tile([S, N * D], f32, tag="sqf")
            nc.scalar.activation(sqf[:], xsqf[:], AF.Square)
            x2f = lda.tile([S, N], f32, tag="x2f")
            nc.vector.tensor_reduce(
                x2f[:], sqf[:].rearrange("t (m d) -> t m d", m=N),
                axis=AX.X, op=OP.add)
            nc.sync.dma_start(x2f_dr.rearrange("m t -> t m"), x2f[:])

            x2B = lda.tile([128, S], f32, tag="x2B")
            for h in range(2):
                nc.sync.dma_start(x2B[h * N:(h + 1) * N, :], x2f_dr[:])

            # inc storage: K in fp32 (4 slots), L in fp16 (12 slots)
            incK = pde.tile([128, 4 * CC * CC], f32, tag="incK")
            incL = pde.tile([128, 12 * CC * CC], f16, tag="incL")

            # differenced paths for the linear-gram increments
            dat, dbt = {}, {}
            for nm in ("y", "z"):
                t = lda.tile([D, NL * CC], f32, tag=f"dat_{nm}")
                av = at[nm][:].rearrange("d (n s) -> d n s", s=S)
                nc.gpsimd.tensor_tensor(
                    t[:].rearrange("d (n c) -> d n c", c=CC),
                    av[:, :, 1:S], av[:, :, 0:S - 1], op=OP.subtract)
                dat[nm] = t
                t2 = lda.tile([D, N * CC], f32, tag=f"dbt_{nm}")
                bv = bt[nm][:].rearrange("d (m t) -> d m t", t=S)
                nc.gpsimd.tensor_tensor(
                    t2[:].rearrange("d (m c) -> d m c", c=CC),
                    bv[:, :, 1:S], bv[:, :, 0:S - 1], op=OP.subtract)
                dbt[nm] = t2

            # ============ gram pipelines ============
            # Scatter DMAs alternate between the SP and Activation HW DGE
            # queues; PSUM->SBUF staging runs on the Pool engine so the
            # Activation queue stays free for PDE coefficient prep.
            qsel = [0]

            def scat_engine():
                qsel[0] += 1
                return nc.sync if (qsel[0] % 2) else nc.scalar

            def emit_kgram_mms():
                for (n0, nn) in NCH:
                    lhsT = at["x"][:, n0 * S:(n0 + nn) * S]
                    for (m0, mw) in MCH:
                        ps = mmp.tile([3 * S, 15 * S], f32, tag="mmK")
                        h, w = nn * S, mw * S
                        nc.tensor.matmul(
                            ps[:h, :w], lhsT,
                            bt["x"][:, m0 * S:(m0 + mw) * S],
                            start=True, stop=True)
                        st = scp.tile([3 * S, 15 * S], f32, tag="mmstK")
                        nc.scalar.copy(st[:h, :w], ps[:h, :w])
                        for j in range(nn):
                            scat_engine().dma_start(
                                gdr0[n0 + j, m0:m0 + mw]
                                .rearrange("m s t -> s m t"),
                                st[j * S:(j + 1) * S, :w]
                                .rearrange("s (m t) -> s m t", t=S))

            def emit_kgram_blocks():
                """gather to pair layout, rbf assembly, increments."""
                for b in range(4):
                    gt = gtp.tile([128, GRID], f32, tag="gt")
                    nc.sync.dma_start(
                        gt[:],
                        gdr0[2 * b:2 * b + 2]
                        .rearrange("h m s t -> (h m) (s t)"))
                    x2A = scp.tile([128, S], f32, tag="x2A")
                    for h in range(2):
                        nc.sync.dma_start(
                            x2A[h * N:(h + 1) * N, :],
                            x2c_dr[2 * b + h:2 * b + h + 1]
                            .broadcast_to((N, S)))
                    u = scp.tile([128, GRID], f32, tag="u")
                    nc.vector.tensor_tensor(
                        u[:].rearrange("p (s t) -> p s t", t=S),
                        x2A[:].rearrange("p (s o) -> p s o", o=1)
                        .broadcast_to((128, S, S)),
                        x2B[:].rearrange("p (o t) -> p o t", o=1)
                        .broadcast_to((128, S, S)),
                        op=OP.add)
                    v = scp.tile([128, GRID], f32, tag="v")
                    nc.vector.scalar_tensor_tensor(
                        v[:], gt[:], 2.0, u[:],
                        op0=OP.mult, op1=OP.subtract)
                    nc.scalar.activation(gt[:], v[:], AF.Exp)
                    # increments: R = G[:,1:]-G[:,:-1]; inc = R[1:,:]-R[:-1,:]
                    gv = gt[:].rearrange("p (s t) -> p s t", t=S)
                    rt = scp.tile([128, S * (S - 1)], f32, tag="rt")
                    rv = rt[:].rearrange("p (s t) -> p s t", t=S - 1)
                    nc.vector.tensor_tensor(
                        rv, gv[:, :, 1:], gv[:, :, :S - 1], op=OP.subtract)
                    nc.vector.tensor_tensor(
                        incK[:, b * CC * CC:(b + 1) * CC * CC]
                        .rearrange("p (a c) -> p a c", c=CC),
                        rv[:, 1:, :], rv[:, :S - 1, :], op=OP.subtract)

            def emit_lgram_mms(g, a_nm, b_nm):
                """differenced-path matmuls -> fp16 increments -> DRAM."""
                for (n0, nn) in NCH:
                    lhsT = dat[a_nm][:, n0 * CC:(n0 + nn) * CC]
                    for (m0, mw) in MCH:
                        ps = mmp.tile([3 * S, 15 * S], f32, tag="mmK")
                        h, w = nn * CC, mw * CC
                        nc.tensor.matmul(
                            ps[:h, :w], lhsT,
                            dbt[b_nm][:, m0 * CC:(m0 + mw) * CC],
                            start=True, stop=True)
                        st = scp.tile([3 * CC, 15 * CC], f16, tag="mmstL")
                        nc.scalar.copy(st[:h, :w], ps[:h, :w])
                        for j in range(nn):
                            scat_engine().dma_start(
                                ldr[g][n0 + j, m0:m0 + mw]
                                .rearrange("m a c -> a m c"),
                                st[j * CC:(j + 1) * CC, :w]
                                .rearrange("a (m c) -> a m c", c=CC))

            def emit_lgram_gathers(g):
                for b in range(4):
                    slot = g * 4 + b
                    nc.sync.dma_start(
                        incL[:, slot * CC * CC:(slot + 1) * CC * CC],
                        ldr[g][2 * b:2 * b + 2]
                        .rearrange("h m a c -> (h m) (a c)"))

            # ============ PDE row-scan solver ============
            # Each spec runs an independent solve on its own engine (DVE or
            # Pool); interleaved emission lets them advance concurrently.
            # Coefficient prep stays on Activation for all solves.
            def emit_pde_multi(specs, hooks=None):
                # spec: (inct, nslots, blk0, nblk, tag, eng, scan_eng) --
                # tensor_tensor_scan only lowers on DVE, so a Pool-hosted
                # solve still issues its scans on the vector engine.
                sts = []
                for (inct, nslots, blk0, nblk, tag, eng, scan_eng) in specs:
                    W = nblk * 65
                    c1s = pde.tile([128, W], f32, tag=f"c1s{tag}")
                    dbuf = pde.tile([128, W], f32, tag=f"d{tag}")
                    eng.memset(c1s[:], 0.0)
                    eng.memset(dbuf[:], 1.0)
                    prev = rwp.tile([128, W], f32, tag=f"row{tag}")
                    eng.memset(prev[:], 1.0)
                    t1 = pde.tile([128, nblk * PQ], f32, tag=f"t1{tag}")
                    t2 = pde.tile([128, nblk * PQ], f32, tag=f"t2{tag}")
                    inc3 = inct[:].rearrange(
                        "p (k a c) -> p k a c",
                        k=nslots, a=CC)[:, blk0:blk0 + nblk]
                    sts.append(dict(nblk=nblk, tag=tag, eng=eng,
                                    scan_eng=scan_eng, c1s=c1s,
                                    dbuf=dbuf, prev=prev, t1=t1, t2=t2,
                                    inc3=inc3, c1b=None, c2b=None))

                for r in range(1, PQ + 1):
                    a = (r - 1) // 2
                    for st_ in sts:
                        nblk, tag, eng = st_["nblk"], st_["tag"], st_["eng"]
                        c1s, dbuf = st_["c1s"], st_["dbuf"]
                        t1, t2 = st_["t1"], st_["t2"]
                        if r % 2 == 1:
                            # JIT coefficients for coarse row a
                            inca = st_["inc3"][:, :, a, :]  # [128, nblk, 32]
                            s12 = cfp.tile([128, nblk * CC], f32,
                                           tag=f"s12{tag}")
                            s12v = s12[:].rearrange("p (b c) -> p b c", c=CC)
                            nc.scalar.activation(s12v, inca, AF.Square,
                                                 scale=SQRT192INV)
                            c2r = cfp.tile([128, nblk * CC], f32,
                                           tag=f"c2r{tag}")
                            nc.scalar.activation(c2r[:], s12[:], AF.Copy,
                                                 scale=-1.0, bias=1.0)
                            vr = cfp.tile([128, nblk * CC], f32,
                                          tag=f"vr{tag}")
                            nc.scalar.activation(
                                vr[:].rearrange("p (b c) -> p b c", c=CC),
                                inca, AF.Copy, scale=0.125, bias=1.0)
                            c1r = cfp.tile([128, nblk * CC], f32,
                                           tag=f"c1r{tag}")
                            eng.tensor_tensor(c1r[:], s12[:], vr[:],
                                              op=OP.add)
                            # stage expanded C1 row (x2 dyadic) into coeffs
                            nc.scalar.activation(
                                c1s[:].rearrange("p (b s) -> p b s", s=65)
                                [:, :, 1:65]
                                .rearrange("p b (c e) -> p b c e", e=2),
                                c1r[:]
                                .rearrange("p (b c o) -> p b c o", c=CC, o=1)
                                .broadcast_to((128, nblk, CC, 2)),
                                AF.Copy)
                            st_["c2b"] = c2r[:].rearrange(
                                "p (b c o) -> p b c o", c=CC, o=1) \
                                .broadcast_to((128, nblk, CC, 2))
                            st_["c1b"] = c1r[:].rearrange(
                                "p (b c o) -> p b c o", c=CC, o=1) \
                                .broadcast_to((128, nblk, CC, 2))

                        pv = st_["prev"][:].rearrange("p (b s) -> p b s", s=65)
                        t1v = t1[:].rearrange("p (b s) -> p b s", s=PQ) \
                            .rearrange("p b (c e) -> p b c e", e=2)
                        t2v = t2[:].rearrange("p (b s) -> p b s", s=PQ) \
                            .rearrange("p b (c e) -> p b c e", e=2)
                        eng.tensor_tensor(
                            t1v,
                            pv[:, :, 1:65]
                            .rearrange("p b (c e) -> p b c e", e=2),
                            st_["c1b"], op=OP.mult)
                        eng.tensor_tensor(
                            t2v,
                            pv[:, :, 0:64]
                            .rearrange("p b (c e) -> p b c e", e=2),
                            st_["c2b"], op=OP.mult)
                        eng.tensor_tensor(
                            dbuf[:].rearrange("p (b s) -> p b s", s=65)
                            [:, :, 1:65],
                            t1[:].rearrange("p (b s) -> p b s", s=PQ),
                            t2[:].rearrange("p (b s) -> p b s", s=PQ),
                            op=OP.subtract)
                        new = rwp.tile([128, st_["nblk"] * 65], f32,
                                       tag=f"row{tag}")
                        st_["scan_eng"].tensor_tensor_scan(
                            new[:], c1s[:], dbuf[:], 1.0,
                            op0=OP.mult, op1=OP.add)
                        st_["prev"] = new
                    if hooks and r in hooks:
                        hooks[r]()
                return [st_["prev"] for st_ in sts]

            # ---- gram matmul/scatter streams: K first, L grams right
            # behind so the DMA queues stay saturated during the K PDE ----
            emit_kgram_mms()
            emit_kgram_blocks()

            # ---- K PDE on DVE ----
            lastK, = emit_pde_multi(
                [(incK, 4, 0, 4, "K", nc.vector, nc.vector)])
            kvals = cst.tile([128, 4], f32, tag="kvals")
            nc.vector.tensor_copy(
                kvals[:].rearrange("p (b o) -> p b o", o=1),
                lastK[:].rearrange("p (b s) -> p b s", s=65)[:, :, 64:65])
            nc.sync.dma_start(ksh_dr.rearrange("b p -> p b"), kvals[:])
            nc.gpsimd.collective_compute(
                "AllGather", mybir.AluOpType.bypass,
                replica_groups=[list(range(N_CORES))],
                ins=[ksh_dr[:]], outs=[kall_dr[:]])
            kt = cst.tile([N, N], f32, tag="kt")
            nc.sync.dma_start(kt[:], kall_dr.rearrange("c b h m -> (c b h) m"))

            # ---- L gram increment streams (PE + Pool + both DMA queues),
            # emitted after the K PDE so K coefficient prep on Act is not
            # queued behind the Act-half of the L scatters ----
            emit_lgram_mms(0, "y", "y")
            emit_lgram_mms(1, "z", "z")
            emit_lgram_mms(2, "y", "z")
            for g in range(3):
                emit_lgram_gathers(g)

            # ---- Newton-Schulz inverse of M = K + I (replicated).
            # GPSIMD cannot touch PSUM, so PSUM consumers run on DVE/Act
            # (tiny ops); emitted via a hook a few rows into the L pair
            # solve so the serial chain overlaps the row recurrences.
            ecols = cst.tile([N, NL], f32, tag="ecols")

            def emit_ns():
                mt = cst.tile([N, N], f32, tag="mt")
                nc.vector.tensor_tensor(mt[:], kt[:], eye[:], op=OP.add)
                r64 = cst.tile([N, 1], f32, tag="r64")
                nc.vector.tensor_reduce(r64[:], mt[:], axis=AX.X, op=OP.add)
                rT = tpp.tile([1, N], f32, tag="tp")
                nc.tensor.transpose(rT[:], r64[:], eye[:])
                rmax = cst.tile([1, 1], f32, tag="rmax")
                nc.vector.tensor_reduce(rmax[:], rT[:], axis=AX.X, op=OP.max)
                alpha = cst.tile([1, 1], f32, tag="alpha")
                nc.vector.reciprocal(alpha[:], rmax[:])
                alps = tpp.tile([N, 1], f32, tag="tp")
                nc.tensor.matmul(alps[:], onesrow[:], alpha[:],
                                 start=True, stop=True)
                alpb = cst.tile([N, 1], f32, tag="alpb")
                nc.scalar.copy(alpb[:], alps[:])
                xns = cst.tile([N, N], f32, tag="xns")
                nc.vector.tensor_scalar_mul(xns[:], eye[:], alpb[:])
                tt = cst.tile([N, N], f32, tag="tt")
                for _ in range(NS_ITERS):
                    p1 = nsp.tile([N, N], f32, tag="ns")
                    nc.tensor.matmul(p1[:], mt[:], xns[:],
                                     start=True, stop=True)
                    nc.vector.scalar_tensor_tensor(
                        tt[:], p1[:], -1.0, twoI[:],
                        op0=OP.mult, op1=OP.add)
                    p2 = nsp.tile([N, N], f32, tag="ns")
                    nc.tensor.matmul(p2[:], xns[:], tt[:],
                                     start=True, stop=True)
                    nc.scalar.copy(xns[:], p2[:])
                # E = B - B^2
                p3 = nsp.tile([N, N], f32, tag="ns")
                nc.tensor.matmul(p3[:], xns[:], xns[:],
                                 start=True, stop=True)
                # et = xns - p3 : (p3 * -1) + xns
                et = cst.tile([N, N], f32, tag="et")
                nc.vector.scalar_tensor_tensor(
                    et[:], p3[:], -1.0, xns[:], op0=OP.mult, op1=OP.add)
                ecp = nsp.tile([N, NL], f32, tag="ns")
                nc.tensor.matmul(ecp[:], et[:], sel[:],
                                 start=True, stop=True)
                nc.scalar.copy(ecols[:], ecp[:])

            # ---- L PDEs: gen+true on DVE concurrently with mix on Pool,
            # Newton-Schulz interleaved after row 6 ----
            lastA, lastB = emit_pde_multi([
                (incL, 12, 0, 5, "LA", nc.vector, nc.vector),
                (incL, 12, 5, 7, "LB", nc.gpsimd, nc.vector),
            ], hooks={6: emit_ns})

            # ---- partial trace ----
            lvals = cst.tile([128, 12], f32, tag="lvals")
            nc.vector.tensor_copy(
                lvals[:, 0:5].rearrange("p (b o) -> p b o", o=1),
                lastA[:].rearrange("p (b s) -> p b s", s=65)[:, :, 64:65])
            nc.vector.tensor_copy(
                lvals[:, 5:12].rearrange("p (b o) -> p b o", o=1),
                lastB[:].rearrange("p (b s) -> p b s", s=65)[:, :, 64:65])
            lsum = cst.tile([128, 4], f32, tag="lsum")
            nc.vector.tensor_tensor(lsum[:], lvals[:, 0:4], lvals[:, 4:8],
                                    op=OP.add)
            nc.vector.scalar_tensor_tensor(
                lsum[:], lvals[:, 8:12], -2.0, lsum[:], op0=OP.mult, op1=OP.add)
            lup_p = tpp.tile([N, 4], f32, tag="tp")
            nc.tensor.matmul(lup_p[:], shf[:], lsum[:], start=True, stop=True)
            lup = cst.tile([N, 4], f32, tag="lup")
            nc.scalar.copy(lup[:], lup_p[:])
            prodA = cst.tile([N, 4], f32, tag="prodA")
            nc.vector.tensor_tensor(
                prodA[:], lsum[0:N, :],
                ecols[:].rearrange("p (c e) -> p c e", e=2)[:, :, 0],
                op=OP.mult)
            prodB = cst.tile([N, 4], f32, tag="prodB")
            nc.vector.tensor_tensor(
                prodB[:], lup[:],
                ecols[:].rearrange("p (c e) -> p c e", e=2)[:, :, 1],
                op=OP.mult)
            ra = cst.tile([N, 1], f32, tag="ra")
            nc.vector.tensor_reduce(ra[:], prodA[:], axis=AX.X, op=OP.add)
            rb = cst.tile([N, 1], f32, tag="rb")
            nc.vector.tensor_reduce(rb[:], prodB[:], axis=AX.X, op=OP.add)
            vsum = cst.tile([N, 1], f32, tag="vsum")
            nc.vector.tensor_tensor(vsum[:], ra[:], rb[:], op=OP.add)
            part = tpp.tile([1, 1], f32, tag="tp")
            nc.tensor.matmul(part[:], vsum[:], ones64[:], start=True, stop=True)
            outst = cst.tile([1, 1], f32, tag="outst")
            nc.scalar.copy(outst[:], part[:])
            nc.sync.dma_start(out_d[:], outst[:])

    nc.compile()
    return nc


def _host_inputs(x, y, z):
    eye = np.eye(N, dtype=np.float32)
    shf = np.zeros((128, N), dtype=np.float32)
    for p in range(N):
        shf[p + N, p] = 1.0
    maps = []
    for c in range(N_CORES):
        sel = np.zeros((N, NL), dtype=np.float32)
        for j in range(NL):
            sel[NL * c + j, j] = 1.0
        maps.append({
            "xf": np.ascontiguousarray(x), "yf": np.ascontiguousarray(y),
            "zf": np.ascontiguousarray(z),
            "xc": np.ascontiguousarray(x[NL * c:NL * (c + 1)]),
            "yc": np.ascontiguousarray(y[NL * c:NL * (c + 1)]),
            "zc": np.ascontiguousarray(z[NL * c:NL * (c + 1)]),
            "eye": eye, "sel": sel, "shf": shf,
        })
    return maps


def _make_runtime():
    """Build the Bass module once and wrap it in a cached jitted callable.

    Per call only x,y,z move host->device (one packed buffer); the
    replication to per-core full copies, the per-core row slices, and the
    donated zero output buffer are all produced on-device by a prep jit.
    Constant inputs (eye/sel/shf) stay resident across calls.
    """
    import jax
    import jax.numpy as jnp
    from jax.experimental.shard_map import shard_map
    from jax.sharding import Mesh, NamedSharding, PartitionSpec
    from concourse import bass2jax, mybir

    nc = _build()
    bass2jax.install_neuronx_cc_hook()

    partition_name = nc.partition_id_tensor.name if nc.partition_id_tensor else None
    in_names, out_names, out_avals = [], [], []
    for alloc in nc.m.functions[0].allocations:
        if not isinstance(alloc, mybir.MemoryLocationSet):
            continue
        name = alloc.memorylocations[0].name
        if alloc.kind == "ExternalInput":
            if name != partition_name:
                in_names.append(name)
        elif alloc.kind == "ExternalOutput":
            out_names.append(name)
            out_avals.append(jax.core.ShapedArray(
                tuple(alloc.tensor_shape), mybir.dt.np(alloc.dtype)))
    n_params = len(in_names)
    in_names_full = in_names + out_names + (
        [partition_name] if partition_name else [])
    donate = tuple(range(n_params, n_params + len(out_names)))

    def _body(*args):
        operands = list(args)
        if partition_name is not None:
            operands.append(bass2jax.partition_id_tensor())
        return tuple(bass2jax._bass_exec_p.bind(
            *operands, out_avals=tuple(out_avals),
            in_names=tuple(in_names_full), out_names=tuple(out_names),
            lowering_input_output_aliases=(),
            sim_require_finite=True, sim_require_nnan=True, nc=nc))

    devices = jax.devices()[:N_CORES]
    mesh = Mesh(np.asarray(devices), ("core",))
    shard = NamedSharding(mesh, PartitionSpec("core"))
    repl = NamedSharding(mesh, PartitionSpec())
    in_specs = (PartitionSpec("core"),) * (n_params + len(out_names))
    out_specs = (PartitionSpec("core"),) * len(out_names)
    sharded = jax.jit(
        shard_map(_body, mesh=mesh, in_specs=in_specs, out_specs=out_specs,
                  check_rep=False),
        donate_argnums=donate, keep_unused=True)

    # constants, staged device-resident once (global = per-core concat)
    eye = np.eye(N, dtype=np.float32)
    shf = np.zeros((128, N), dtype=np.float32)
    for p in range(N):
        shf[p + N, p] = 1.0
    sel_g = np.zeros((N_CORES, N, NL), np.float32)
    for c in range(N_CORES):
        for j in range(NL):
            sel_g[c, NL * c + j, j] = 1.0
    const = {
        "eye": jax.device_put(np.tile(eye, (N_CORES, 1)), shard),
        "sel": jax.device_put(sel_g.reshape(N_CORES * N, NL), shard),
        "shf": jax.device_put(np.tile(shf, (N_CORES, 1)), shard),
    }
    jax.block_until_ready(list(const.values()))

    def _prep(packed):
        x, y, z = packed[0], packed[1], packed[2]
        xf = jnp.tile(x, (N_CORES, 1, 1))
        yf = jnp.tile(y, (N_CORES, 1, 1))
        zf = jnp.tile(z, (N_CORES, 1, 1))
        zer = jnp.zeros((N_CORES, 1, 1), jnp.float32)
        return xf, yf, zf, x, y, z, zer

    prep = jax.jit(_prep, in_shardings=(repl,), out_shardings=(shard,) * 7)
    name2idx = {nm: i for i, nm in enumerate(in_names)}

    def call(x, y, z):
        packed = np.stack([x, y, z])
        xf, yf, zf, xc, yc, zc, zer = prep(packed)
        args = [None] * n_params
        args[name2idx["xf"]] = xf
        args[name2idx["yf"]] = yf
        args[name2idx["zf"]] = zf
        args[name2idx["xc"]] = xc
        args[name2idx["yc"]] = yc
        args[name2idx["zc"]] = zc
        for nm, buf in const.items():
            args[name2idx[nm]] = buf
        out = sharded(*args, zer)
        vals = np.asarray(out[0]).reshape(N_CORES)
        return np.float32(np.float64(vals).sum())

    return call


def _kernel_fallback(x, y, z):
    from concourse import bass_utils
    if "nc" not in _CACHE:
        _CACHE["nc"] = _build()
    nc = _CACHE["nc"]
    maps = _host_inputs(x, y, z)
    res = bass_utils.run_bass_kernel_spmd(nc, maps, core_ids=list(range(N_CORES)))
    total = np.float64(0.0)
    for c in range(N_CORES):
        total += np.float64(res.results[c]["out"][0, 0])
    return np.float32(total)


def kernel(x, y, z):
    x = np.ascontiguousarray(np.asarray(x, np.float32))
    y = np.ascontiguousarray(np.asarray(y, np.float32))
    z = np.ascontiguousarray(np.asarray(z, np.float32))
    if not _CACHE.get("rt_failed"):
        # retry once with a fresh runtime: a transient device error
        # (e.g. NRT_EXEC_UNIT_UNRECOVERABLE) usually clears on re-run
        for _ in range(2):
            try:
                if "rt" not in _CACHE:
                    _CACHE["rt"] = _make_runtime()
                return _CACHE["rt"](x, y, z)
            except Exception:
                _CACHE.pop("rt", None)
        _CACHE["rt_failed"] = True
    return _kernel_fallback(x, y, z)

